# revision 25
# baseline (speedup 1.0000x reference)
"""Trainium2 Bass kernel for sparse-projection + WTA top-k masking.

Computes out = topk_mask_32(input @ W.T) where W [10240, 512] is built from
per-row COO entries (weight_vals/weight_idx, duplicates accumulate).

Strategy (hardcoded for B=4096, F=512, O=10240, K=32, 8 cores):
  - Shard the OUTPUT dim across cores (O-shard): each core computes the full
    batch against a 1280-column slice of W.T.  Per-core DMA: 2MB input +
    0.6MB weights in (fp8), 10MB scores out (fp16) -- well under the PE.
  - Device (SPMD x8): fp8(e4m3) matmuls in DoubleRow perf mode (2 weights
    per PE cell -> 256-deep contraction, 2 streamed elements/cycle): 160 MMs
    of [128o x 512b x 256k] at ~216ns = ~35us of PE, half the fp16 dense
    cost.  Operands live in [128, 4, dim] contraction-subtile layout; a MM
    consumes a k-subtile PAIR of the stationary wt slice and the moving
    input.  Loop runs QUARTER passes (one 1024-wide b-superchunk across all
    10 o-tiles) so pass 0 only needs ~0.7MB before the PE saturates; input
    streams on sync strictly in consumption order, weights on scalar (o-tile
    0 slice first).  8 dummy MMs on a zeroed tile warm the HAM clock gate
    (1.2->2.4GHz) during the ~4us DGE first-transfer latency.  PSUM [128,
    1024] tiles rotate 4-deep; evictions (fp32->fp16) alternate ACT/DVE by
    o-tile parity (each ~25us busy; loading one engine with all of them
    trips the chip-wide P0 power downclock), stores alternate the two HWDGE
    rings, and the final iteration splits evict+store across both to cut the
    tail.  No on-device top-k: the old max8/find_index8 pipeline put ~100us
    on the DVE; shipping fp16 scores costs ~29us of overlapped DMA instead.
  - Host: top-128 candidates per row from the fp16 scores (argpartition),
    EXACT recompute of those via the 32-entry COO rows (vectorized gather,
    no GEMM), exact top-32, scatter.  fp8 score noise (rms ~0.05, max ~0.5)
    cannot demote a true top-32 element past approx rank 128 (the
    rank-32..160 value spread is ~2.5), so no rescue pass is needed and
    output values are exact fp32.
"""

import numpy as np
import concourse.bacc as bacc
import concourse.bass as bass
import concourse.tile as tile
import concourse.mybir as mybir
from concourse.bass_utils import run_bass_kernel_spmd

F32 = mybir.dt.float32
F16 = mybir.dt.float16
F8 = mybir.dt.float8e4

B = 4096          # batch
F = 512           # in_features
O = 10240         # out_features
TOPK = 32
NCORES = 8
OL = O // NCORES  # 1280 output cols per core
OT = OL // 128    # 10 o-tiles per core
KT = F // 128     # 4 k-tiles
NB = 512          # b-chunk width (moving operand / one PSUM bank fp32)
BCH = B // NB     # 8 b-chunks
NCAND = 128       # host-side candidate count per row
N_WARM = 8        # HAM warm-up dummy matmuls


def build_program() -> bass.Bass:
    nc = bacc.Bacc()
    inT = nc.declare_dram_parameter("inT", [128, KT, B], F8, isOutput=False)
    wt = nc.declare_dram_parameter("wt", [128, KT, OL], F8, isOutput=False)
    x_d = nc.declare_dram_parameter("x", [OL, B], F16, isOutput=True)

    with tile.TileContext(nc) as tc:
        with (
            tc.tile_pool(name="insb", bufs=1) as inpool,
            tc.tile_pool(name="wtsb", bufs=1) as wtpool,
            tc.tile_pool(name="psum", bufs=4, space=bass.MemorySpace.PSUM) as pspool,
            tc.tile_pool(name="xout", bufs=6) as xpool,
        ):
            insb = inpool.tile([128, KT, B], F8, name="insb", tag="insb")
            wtsb = wtpool.tile([128, KT, OL], F8, name="wtsb", tag="wtsb")

            # HAM warm-up: the PE clock sits at 1.2GHz until ~3.4us of
            # sustained activity.  Burn the DMA-latency window (engines start
            # ~6.6us, first data lands ~8.5us) with dummy MMs on a zeroed
            # tile so the real MMs run at 2.4GHz from the start.
            warm = inpool.tile([128, NB], F16, name="warm", tag="warm")
            nc.vector.memset(warm[:], 0)
            # the warm tile shares the "ps" rotation (buf 0); its first reuse
            # is iteration 3, long after the dummies retire
            wps = pspool.tile([128, 2 * NB], F32, name="wps", tag="ps")
            # 8 cold MMs = ~3.4us of continuous PE busy = one full HAM
            # window, so K=8/8 fires right as the first real data lands
            # (~3us DGE first-transfer latency + stream); queueing the real
            # MMs behind the dummies costs nothing
            for _ in range(N_WARM):
                nc.tensor.matmul(wps[:, 0:NB], warm[:, 0:128], warm[:],
                                 start=True, stop=True)

            # only sync+scalar are HWDGE engines: weights stream on scalar
            # (idle until its first eviction), input on sync in 1024-wide
            # (2KB-line) b-superchunks.  The MM loop runs QUARTER passes (one
            # 1024-wide b-superchunk across all 10 o-tiles, ~19us of PE work
            # each), so pass 0 only needs wt + superchunk 0 = 2.25MB before
            # the PE saturates, and the input stream (strictly in
            # consumption order) stays ~one pass ahead thereafter.
            # wt k-tiles split: the 128-col slice o-tile 0 needs lands in
            # ~0.5us so the PE starts at ~7.5us instead of waiting for the
            # full 1.25MB of weights
            nc.scalar.dma_start(wtsb[:, :, 0:128], wt[:, :, 0:128])
            nc.scalar.dma_start(wtsb[:, :, 128:OL], wt[:, :, 128:OL])
            for sc in range(B // 1024):
                for k in range(KT):
                    nc.sync.dma_start(
                        insb[:, k, sc * 1024:(sc + 1) * 1024],
                        inT[:, k, sc * 1024:(sc + 1) * 1024])

            for sc in range(B // 1024):
                for ot in range(OT):
                    ps = pspool.tile([128, 2 * NB], F32, name="ps", tag="ps")
                    for kk in range(KT // 2):
                        ksl = slice(2 * kk, 2 * kk + 2)
                        for j in range(2):
                            b = 2 * sc + j
                            nc.tensor.matmul(
                                ps[:, j * NB:(j + 1) * NB],
                                wtsb[:, ksl, ot * 128:(ot + 1) * 128],
                                insb[:, ksl, b * NB:(b + 1) * NB],
                                start=(kk == 0),
                                stop=(kk == KT // 2 - 1),
                                perf_mode=mybir.MatmulPerfMode.DoubleRow,
                            )
                    xh = xpool.tile([128, 2 * NB], F16, name="xh", tag="xh")
                    orow = x_d[ot * 128:(ot + 1) * 128,
                               sc * 1024:(sc + 1) * 1024]
                    # evictions split ACT/DVE by o-tile parity: each engine
                    # ~25us busy; running them all on one engine pushed the
                    # chip into the sustained-power P0 downclock (2.0GHz on
                    # every engine, 5/5 runs) and cost ~15us overall
                    if sc == 3 and ot == OT - 1:
                        # final iteration: halve evict+store across engines/
                        # rings so the tail after the last MM shrinks
                        nc.scalar.copy(xh[:, 0:NB], ps[:, 0:NB])
                        nc.vector.tensor_copy(xh[:, NB:2 * NB], ps[:, NB:2 * NB])
                        nc.scalar.dma_start(orow[:, 0:NB], xh[:, 0:NB])
                        nc.sync.dma_start(orow[:, NB:2 * NB], xh[:, NB:2 * NB])
                    elif ot % 2 == 0:
                        nc.scalar.copy(xh[:], ps[:])
                        nc.scalar.dma_start(orow, xh[:])
                    else:
                        nc.vector.tensor_copy(xh[:], ps[:])
                        nc.sync.dma_start(orow, xh[:])
    nc.compile()
    return nc


_NC = None


def _get_program() -> bass.Bass:
    global _NC
    if _NC is None:
        _NC = build_program()
    return _NC


# host-side context for gather_output's exact candidate recompute
_CTX = {}


def prepare_in_maps(input, weight_vals, weight_idx):
    input = np.ascontiguousarray(np.asarray(input, dtype=np.float32))
    weight_vals = np.asarray(weight_vals, dtype=np.float32)
    weight_idx = np.asarray(weight_idx).astype(np.int64)

    # Dense W on host (COO duplicates add), transposed + fp16 for the device.
    import ml_dtypes
    F8NP = ml_dtypes.float8_e4m3
    W = np.zeros((O, F), dtype=np.float32)
    np.add.at(W, (np.arange(O)[:, None], weight_idx), weight_vals)
    # [128, KT, dim] contraction-subtile layout for DoubleRow fp8 matmuls
    wt3 = np.ascontiguousarray(
        W.T.reshape(KT, 128, O).transpose(1, 0, 2).astype(F8NP))
    in3 = np.ascontiguousarray(
        input.T.reshape(KT, 128, B).transpose(1, 0, 2).astype(F8NP))

    _CTX["input"] = input
    _CTX["weight_vals"] = weight_vals
    _CTX["weight_idx"] = weight_idx

    return [
        {"inT": in3,
         "wt": np.ascontiguousarray(wt3[:, :, c * OL:(c + 1) * OL])}
        for c in range(NCORES)
    ]


def gather_output(results) -> np.ndarray:
    input = _CTX["input"]
    weight_vals = _CTX["weight_vals"]
    weight_idx = _CTX["weight_idx"]

    X = np.concatenate(
        [np.asarray(results[c]["x"]) for c in range(NCORES)], axis=0)  # [O, B]
    S = X.T.astype(np.float32)                                         # [B, O]

    # approx top-64 per row, then exact recompute of just those candidates
    # via the 32-entry COO rows (sum_p vals[o,p] * input[b, idx[o,p]])
    cand = np.argpartition(-S, NCAND - 1, axis=1)[:, :NCAND]   # [B, 64]
    rows = np.arange(B)[:, None]
    wi = weight_idx[cand]                                      # [B, 64, 32]
    wv = weight_vals[cand].astype(np.float64)                  # [B, 64, 32]
    xg = input[rows[:, :, None], wi]                           # [B, 64, 32]
    exact = (wv * xg).sum(axis=2)                              # [B, 64] f64

    # exact top-32 of the 64 (desc value, ties by lower column like top_k)
    order = np.lexsort((cand, -exact), axis=1)[:, :TOPK]
    g32 = np.take_along_axis(cand, order, axis=1)
    v32 = np.take_along_axis(exact, order, axis=1).astype(np.float32)

    out = np.zeros((B, O), dtype=np.float32)
    out[rows, g32] = v32
    return out


def kernel(input, weight_vals, weight_idx):
    in_maps = prepare_in_maps(input, weight_vals, weight_idx)
    res = run_bass_kernel_spmd(_get_program(), in_maps, list(range(NCORES)))
    return gather_output(res.results)


# revision 27
# speedup vs baseline: 1.0866x; 1.0866x over previous
"""Trainium2 Bass kernel for sparse-projection + WTA top-k masking.

Computes out = topk_mask_32(input @ W.T) where W [10240, 512] is built from
per-row COO entries (weight_vals/weight_idx, duplicates accumulate).

Strategy (hardcoded for B=4096, F=512, O=10240, K=32, 8 cores):
  - Shard the OUTPUT dim across cores (O-shard): each core computes the full
    batch against a 1280-column slice of W.T.  Per-core DMA: 2MB input +
    0.6MB weights in (fp8), 10MB scores out (fp16) -- well under the PE.
  - Device (SPMD x8): fp8(e4m3) matmuls in DoubleRow perf mode (2 weights
    per PE cell -> 256-deep contraction, 2 streamed elements/cycle): 160 MMs
    of [128o x 512b x 256k] at ~216ns = ~35us of PE, half the fp16 dense
    cost.  Operands live in [128, 4, dim] contraction-subtile layout; a MM
    consumes a k-subtile PAIR of the stationary wt slice and the moving
    input.  Loop runs QUARTER passes (one 1024-wide b-superchunk across all
    10 o-tiles) so pass 0 only needs ~0.7MB before the PE saturates; input
    streams on sync strictly in consumption order, weights on scalar (o-tile
    0 slice first).  8 dummy MMs on a zeroed tile warm the HAM clock gate
    (1.2->2.4GHz) during the ~4us DGE first-transfer latency.  PSUM [128,
    1024] tiles rotate 4-deep; evictions (fp32->fp16) alternate ACT/DVE by
    o-tile parity (each ~25us busy; loading one engine with all of them
    trips the chip-wide P0 power downclock), stores alternate the two HWDGE
    rings, and the final iteration splits evict+store across both to cut the
    tail.  No on-device top-k: the old max8/find_index8 pipeline put ~100us
    on the DVE; shipping fp16 scores costs ~29us of overlapped DMA instead.
  - Host: top-128 candidates per row from the fp16 scores (argpartition),
    EXACT recompute of those via the 32-entry COO rows (vectorized gather,
    no GEMM), exact top-32, scatter.  fp8 score noise (rms ~0.05, max ~0.5)
    cannot demote a true top-32 element past approx rank 128 (the
    rank-32..160 value spread is ~2.5), so no rescue pass is needed and
    output values are exact fp32.
"""

import numpy as np
import concourse.bacc as bacc
import concourse.bass as bass
import concourse.tile as tile
import concourse.mybir as mybir
from concourse.bass_utils import run_bass_kernel_spmd

F32 = mybir.dt.float32
F16 = mybir.dt.float16
F8 = mybir.dt.float8e4

B = 4096          # batch
F = 512           # in_features
O = 10240         # out_features
TOPK = 32
NCORES = 8
OL = O // NCORES  # 1280 output cols per core
OT = OL // 128    # 10 o-tiles per core
KT = F // 128     # 4 k-tiles
NB = 512          # b-chunk width (moving operand / one PSUM bank fp32)
BCH = B // NB     # 8 b-chunks
NCAND = 128       # host-side candidate count per row
N_WARM = 8        # HAM warm-up dummy matmuls


def build_program() -> bass.Bass:
    nc = bacc.Bacc()
    inT = nc.declare_dram_parameter("inT", [128, KT, B], F8, isOutput=False)
    wt = nc.declare_dram_parameter("wt", [128, KT, OL], F8, isOutput=False)
    x_d = nc.declare_dram_parameter("x", [OL, B], F16, isOutput=True)

    with tile.TileContext(nc) as tc:
        with (
            tc.tile_pool(name="insb", bufs=1) as inpool,
            tc.tile_pool(name="wtsb", bufs=1) as wtpool,
            tc.tile_pool(name="psum", bufs=4, space=bass.MemorySpace.PSUM) as pspool,
            tc.tile_pool(name="xout", bufs=6) as xpool,
        ):
            insb = inpool.tile([128, KT, B], F8, name="insb", tag="insb")
            wtsb = wtpool.tile([128, KT, OL], F8, name="wtsb", tag="wtsb")

            # HAM warm-up: the PE clock sits at 1.2GHz until ~3.4us of
            # sustained activity.  Burn the DMA-latency window (engines start
            # ~6.6us, first data lands ~8.5us) with dummy MMs on a zeroed
            # tile so the real MMs run at 2.4GHz from the start.
            warm = inpool.tile([128, NB], F16, name="warm", tag="warm")
            nc.vector.memset(warm[:], 0)
            # the warm tile shares the "ps" rotation (buf 0); its first reuse
            # is iteration 3, long after the dummies retire
            wps = pspool.tile([128, 2 * NB], F32, name="wps", tag="ps")
            # 8 cold MMs = ~3.4us of continuous PE busy = one full HAM
            # window, so K=8/8 fires right as the first real data lands
            # (~3us DGE first-transfer latency + stream); queueing the real
            # MMs behind the dummies costs nothing
            for _ in range(N_WARM):
                nc.tensor.matmul(wps[:, 0:NB], warm[:, 0:128], warm[:],
                                 start=True, stop=True)

            # only sync+scalar are HWDGE engines: weights stream on scalar
            # (idle until its first eviction), input on sync in 1024-wide
            # (2KB-line) b-superchunks.  The MM loop runs QUARTER passes (one
            # 1024-wide b-superchunk across all 10 o-tiles, ~19us of PE work
            # each), so pass 0 only needs wt + superchunk 0 = 2.25MB before
            # the PE saturates, and the input stream (strictly in
            # consumption order) stays ~one pass ahead thereafter.
            # wt k-tiles split: the 128-col slice o-tile 0 needs lands in
            # ~0.5us so the PE starts at ~7.5us instead of waiting for the
            # full 1.25MB of weights
            nc.scalar.dma_start(wtsb[:, :, 0:128], wt[:, :, 0:128])
            nc.scalar.dma_start(wtsb[:, :, 128:OL], wt[:, :, 128:OL])
            # input in 2048-wide chunks (2KB lines, ~405GB/s vs 183 at 1KB):
            # ~5.5us of sync instead of 11, leaving room for all 40 stores
            for scH in range(2):
                for k in range(KT):
                    nc.sync.dma_start(
                        insb[:, k, scH * 2048:(scH + 1) * 2048],
                        inT[:, k, scH * 2048:(scH + 1) * 2048])

            for sc in range(B // 1024):
                for ot in range(OT):
                    ps = pspool.tile([128, 2 * NB], F32, name="ps", tag="ps")
                    for kk in range(KT // 2):
                        ksl = slice(2 * kk, 2 * kk + 2)
                        for j in range(2):
                            b = 2 * sc + j
                            nc.tensor.matmul(
                                ps[:, j * NB:(j + 1) * NB],
                                wtsb[:, ksl, ot * 128:(ot + 1) * 128],
                                insb[:, ksl, b * NB:(b + 1) * NB],
                                start=(kk == 0),
                                stop=(kk == KT // 2 - 1),
                                perf_mode=mybir.MatmulPerfMode.DoubleRow,
                            )
                    xh = xpool.tile([128, 2 * NB], F16, name="xh", tag="xh")
                    orow = x_d[ot * 128:(ot + 1) * 128,
                               sc * 1024:(sc + 1) * 1024]
                    # evictions split ACT/DVE by o-tile parity (running all
                    # on one engine trips the chip-wide P0 power downclock);
                    # ALL steady-state stores on sync: an inline DMA
                    # descriptor costs its issuing engine ~0.6us, and
                    # evict+store on scalar saturated it (98% of the 1.73us
                    # pair cadence), stalling the PSUM rotation
                    if sc == 3 and ot == OT - 1:
                        # final iteration: halve evict+store across engines/
                        # rings so the tail after the last MM shrinks
                        nc.scalar.copy(xh[:, 0:NB], ps[:, 0:NB])
                        nc.vector.tensor_copy(xh[:, NB:2 * NB], ps[:, NB:2 * NB])
                        nc.scalar.dma_start(orow[:, 0:NB], xh[:, 0:NB])
                        nc.sync.dma_start(orow[:, NB:2 * NB], xh[:, NB:2 * NB])
                    else:
                        if ot % 2 == 0:
                            nc.scalar.copy(xh[:], ps[:])
                        else:
                            nc.vector.tensor_copy(xh[:], ps[:])
                        nc.sync.dma_start(orow, xh[:])
    nc.compile()
    return nc


_NC = None


def _get_program() -> bass.Bass:
    global _NC
    if _NC is None:
        _NC = build_program()
    return _NC


# host-side context for gather_output's exact candidate recompute
_CTX = {}


def prepare_in_maps(input, weight_vals, weight_idx):
    input = np.ascontiguousarray(np.asarray(input, dtype=np.float32))
    weight_vals = np.asarray(weight_vals, dtype=np.float32)
    weight_idx = np.asarray(weight_idx).astype(np.int64)

    # Dense W on host (COO duplicates add), transposed + fp16 for the device.
    import ml_dtypes
    F8NP = ml_dtypes.float8_e4m3
    W = np.zeros((O, F), dtype=np.float32)
    np.add.at(W, (np.arange(O)[:, None], weight_idx), weight_vals)
    # [128, KT, dim] contraction-subtile layout for DoubleRow fp8 matmuls
    wt3 = np.ascontiguousarray(
        W.T.reshape(KT, 128, O).transpose(1, 0, 2).astype(F8NP))
    in3 = np.ascontiguousarray(
        input.T.reshape(KT, 128, B).transpose(1, 0, 2).astype(F8NP))

    _CTX["input"] = input
    _CTX["weight_vals"] = weight_vals
    _CTX["weight_idx"] = weight_idx

    return [
        {"inT": in3,
         "wt": np.ascontiguousarray(wt3[:, :, c * OL:(c + 1) * OL])}
        for c in range(NCORES)
    ]


def gather_output(results) -> np.ndarray:
    input = _CTX["input"]
    weight_vals = _CTX["weight_vals"]
    weight_idx = _CTX["weight_idx"]

    X = np.concatenate(
        [np.asarray(results[c]["x"]) for c in range(NCORES)], axis=0)  # [O, B]
    S = X.T.astype(np.float32)                                         # [B, O]

    # approx top-64 per row, then exact recompute of just those candidates
    # via the 32-entry COO rows (sum_p vals[o,p] * input[b, idx[o,p]])
    cand = np.argpartition(-S, NCAND - 1, axis=1)[:, :NCAND]   # [B, 64]
    rows = np.arange(B)[:, None]
    wi = weight_idx[cand]                                      # [B, 64, 32]
    wv = weight_vals[cand].astype(np.float64)                  # [B, 64, 32]
    xg = input[rows[:, :, None], wi]                           # [B, 64, 32]
    exact = (wv * xg).sum(axis=2)                              # [B, 64] f64

    # exact top-32 of the 64 (desc value, ties by lower column like top_k)
    order = np.lexsort((cand, -exact), axis=1)[:, :TOPK]
    g32 = np.take_along_axis(cand, order, axis=1)
    v32 = np.take_along_axis(exact, order, axis=1).astype(np.float32)

    out = np.zeros((B, O), dtype=np.float32)
    out[rows, g32] = v32
    return out


def kernel(input, weight_vals, weight_idx):
    in_maps = prepare_in_maps(input, weight_vals, weight_idx)
    res = run_bass_kernel_spmd(_get_program(), in_maps, list(range(NCORES)))
    return gather_output(res.results)


# revision 29
# speedup vs baseline: 1.1482x; 1.0566x over previous
"""Trainium2 Bass kernel for sparse-projection + WTA top-k masking.

Computes out = topk_mask_32(input @ W.T) where W [10240, 512] is built from
per-row COO entries (weight_vals/weight_idx, duplicates accumulate).

Strategy (hardcoded for B=4096, F=512, O=10240, K=32, 8 cores):
  - Shard the OUTPUT dim across cores (O-shard): each core computes the full
    batch against a 1280-column slice of W.T.  Per-core DMA: 2MB input +
    0.6MB weights in (fp8), 10MB scores out (fp16) -- well under the PE.
  - Device (SPMD x8): fp8(e4m3) matmuls in DoubleRow perf mode (2 weights
    per PE cell -> 256-deep contraction, 2 streamed elements/cycle): 160 MMs
    of [128o x 512b x 256k] at ~216ns = ~35us of PE, half the fp16 dense
    cost.  Operands live in [128, 4, dim] contraction-subtile layout; a MM
    consumes a k-subtile PAIR of the stationary wt slice and the moving
    input.  Loop runs QUARTER passes (one 1024-wide b-superchunk across all
    10 o-tiles) so pass 0 only needs ~0.7MB before the PE saturates; input
    streams on sync strictly in consumption order, weights on scalar (o-tile
    0 slice first).  8 dummy MMs on a zeroed tile warm the HAM clock gate
    (1.2->2.4GHz) during the ~4us DGE first-transfer latency.  PSUM [128,
    1024] tiles rotate 4-deep; evictions (fp32->fp16) alternate ACT/DVE by
    o-tile parity (each ~25us busy; loading one engine with all of them
    trips the chip-wide P0 power downclock), stores alternate the two HWDGE
    rings, and the final iteration splits evict+store across both to cut the
    tail.  No on-device top-k: the old max8/find_index8 pipeline put ~100us
    on the DVE; shipping fp16 scores costs ~29us of overlapped DMA instead.
  - Host: top-128 candidates per row from the fp16 scores (argpartition),
    EXACT recompute of those via the 32-entry COO rows (vectorized gather,
    no GEMM), exact top-32, scatter.  fp8 score noise (rms ~0.05, max ~0.5)
    cannot demote a true top-32 element past approx rank 128 (the
    rank-32..160 value spread is ~2.5), so no rescue pass is needed and
    output values are exact fp32.
"""

import numpy as np
import concourse.bacc as bacc
import concourse.bass as bass
import concourse.tile as tile
import concourse.mybir as mybir
from concourse.bass_utils import run_bass_kernel_spmd

F32 = mybir.dt.float32
F16 = mybir.dt.float16
F8 = mybir.dt.float8e4

B = 4096          # batch
F = 512           # in_features
O = 10240         # out_features
TOPK = 32
NCORES = 8
OL = O // NCORES  # 1280 output cols per core
OT = OL // 128    # 10 o-tiles per core
KT = F // 128     # 4 k-tiles
NB = 512          # b-chunk width (moving operand / one PSUM bank fp32)
BCH = B // NB     # 8 b-chunks
NCAND = 128       # host-side candidate count per row
N_WARM = 8        # HAM warm-up dummy matmuls


def build_program() -> bass.Bass:
    nc = bacc.Bacc()
    inT = nc.declare_dram_parameter("inT", [128, KT, B], F8, isOutput=False)
    wt = nc.declare_dram_parameter("wt", [128, KT, OL], F8, isOutput=False)
    x_d = nc.declare_dram_parameter("x", [OL, B], F16, isOutput=True)

    with tile.TileContext(nc) as tc:
        with (
            tc.tile_pool(name="insb", bufs=1) as inpool,
            tc.tile_pool(name="wtsb", bufs=1) as wtpool,
            tc.tile_pool(name="psum", bufs=4, space=bass.MemorySpace.PSUM) as pspool,
            tc.tile_pool(name="xout", bufs=6) as xpool,
        ):
            insb = inpool.tile([128, KT, B], F8, name="insb", tag="insb")
            wtsb = wtpool.tile([128, KT, OL], F8, name="wtsb", tag="wtsb")

            # HAM warm-up: the PE clock sits at 1.2GHz until ~3.4us of
            # sustained activity.  Burn the DMA-latency window (engines start
            # ~6.6us, first data lands ~8.5us) with dummy MMs on a zeroed
            # tile so the real MMs run at 2.4GHz from the start.
            warm = inpool.tile([128, NB], F16, name="warm", tag="warm")
            nc.vector.memset(warm[:], 0)
            # the warm tile shares the "ps" rotation (buf 0); its first reuse
            # is iteration 3, long after the dummies retire
            wps = pspool.tile([128, 2 * NB], F32, name="wps", tag="ps")
            # 8 cold MMs = ~3.4us of continuous PE busy = one full HAM
            # window, so K=8/8 fires right as the first real data lands
            # (~3us DGE first-transfer latency + stream); queueing the real
            # MMs behind the dummies costs nothing
            for _ in range(N_WARM):
                nc.tensor.matmul(wps[:, 0:NB], warm[:, 0:128], warm[:],
                                 start=True, stop=True)

            # only sync+scalar are HWDGE engines: weights stream on scalar
            # (idle until its first eviction), input on sync in 1024-wide
            # (2KB-line) b-superchunks.  The MM loop runs QUARTER passes (one
            # 1024-wide b-superchunk across all 10 o-tiles, ~19us of PE work
            # each), so pass 0 only needs wt + superchunk 0 = 2.25MB before
            # the PE saturates, and the input stream (strictly in
            # consumption order) stays ~one pass ahead thereafter.
            # wt k-tiles split: the 128-col slice o-tile 0 needs lands in
            # ~0.5us so the PE starts at ~7.5us instead of waiting for the
            # full 1.25MB of weights
            nc.scalar.dma_start(wtsb[:, :, 0:128], wt[:, :, 0:128])
            nc.scalar.dma_start(wtsb[:, :, 128:OL], wt[:, :, 128:OL])
            # pass-0 chunks narrow (128KB) so the first k-groups land with
            # minimum latency and the ramp has no HAM-re-throttling gaps;
            # the back half in 2048-wide chunks (2KB lines, ~405GB/s)
            for k in range(KT):
                nc.sync.dma_start(insb[:, k, 0:1024], inT[:, k, 0:1024])
            for k in range(KT):
                nc.sync.dma_start(insb[:, k, 1024:2048], inT[:, k, 1024:2048])
            for k in range(KT):
                nc.sync.dma_start(insb[:, k, 2048:B], inT[:, k, 2048:B])

            for sc in range(B // 1024):
                for ot in range(OT):
                    ps = pspool.tile([128, 2 * NB], F32, name="ps", tag="ps")
                    # final iteration runs j-OUTER so its first PSUM half
                    # completes 2 MMs early and the half-eviction overlaps
                    # the last MMs (shaves ~1us of tail); everywhere else
                    # kk-outer reuses the stationary operand across both j
                    last = (sc == 3 and ot == OT - 1)
                    loop = ([(kk, j) for j in range(2) for kk in range(KT // 2)]
                            if last else
                            [(kk, j) for kk in range(KT // 2) for j in range(2)])
                    for kk, j in loop:
                        ksl = slice(2 * kk, 2 * kk + 2)
                        b = 2 * sc + j
                        nc.tensor.matmul(
                            ps[:, j * NB:(j + 1) * NB],
                            wtsb[:, ksl, ot * 128:(ot + 1) * 128],
                            insb[:, ksl, b * NB:(b + 1) * NB],
                            start=(kk == 0),
                            stop=(kk == KT // 2 - 1),
                            perf_mode=mybir.MatmulPerfMode.DoubleRow,
                        )
                    xh = xpool.tile([128, 2 * NB], F16, name="xh", tag="xh")
                    orow = x_d[ot * 128:(ot + 1) * 128,
                               sc * 1024:(sc + 1) * 1024]
                    # evictions split ACT/DVE by o-tile parity (running all
                    # on one engine trips the chip-wide P0 power downclock);
                    # ALL steady-state stores on sync: an inline DMA
                    # descriptor costs its issuing engine ~0.6us, and
                    # evict+store on scalar saturated it (98% of the 1.73us
                    # pair cadence), stalling the PSUM rotation
                    if sc == 3 and ot == OT - 1:
                        # final iteration: halve evict+store across engines/
                        # rings so the tail after the last MM shrinks
                        nc.scalar.copy(xh[:, 0:NB], ps[:, 0:NB])
                        nc.vector.tensor_copy(xh[:, NB:2 * NB], ps[:, NB:2 * NB])
                        nc.scalar.dma_start(orow[:, 0:NB], xh[:, 0:NB])
                        nc.sync.dma_start(orow[:, NB:2 * NB], xh[:, NB:2 * NB])
                    else:
                        if ot % 2 == 0:
                            nc.scalar.copy(xh[:], ps[:])
                        else:
                            nc.vector.tensor_copy(xh[:], ps[:])
                        nc.sync.dma_start(orow, xh[:])
    nc.compile()
    return nc


_NC = None


def _get_program() -> bass.Bass:
    global _NC
    if _NC is None:
        _NC = build_program()
    return _NC


# host-side context for gather_output's exact candidate recompute
_CTX = {}


def prepare_in_maps(input, weight_vals, weight_idx):
    input = np.ascontiguousarray(np.asarray(input, dtype=np.float32))
    weight_vals = np.asarray(weight_vals, dtype=np.float32)
    weight_idx = np.asarray(weight_idx).astype(np.int64)

    # Dense W on host (COO duplicates add), transposed + fp16 for the device.
    import ml_dtypes
    F8NP = ml_dtypes.float8_e4m3
    W = np.zeros((O, F), dtype=np.float32)
    np.add.at(W, (np.arange(O)[:, None], weight_idx), weight_vals)
    # [128, KT, dim] contraction-subtile layout for DoubleRow fp8 matmuls
    wt3 = np.ascontiguousarray(
        W.T.reshape(KT, 128, O).transpose(1, 0, 2).astype(F8NP))
    in3 = np.ascontiguousarray(
        input.T.reshape(KT, 128, B).transpose(1, 0, 2).astype(F8NP))

    _CTX["input"] = input
    _CTX["weight_vals"] = weight_vals
    _CTX["weight_idx"] = weight_idx

    return [
        {"inT": in3,
         "wt": np.ascontiguousarray(wt3[:, :, c * OL:(c + 1) * OL])}
        for c in range(NCORES)
    ]


def gather_output(results) -> np.ndarray:
    input = _CTX["input"]
    weight_vals = _CTX["weight_vals"]
    weight_idx = _CTX["weight_idx"]

    X = np.concatenate(
        [np.asarray(results[c]["x"]) for c in range(NCORES)], axis=0)  # [O, B]
    S = X.T.astype(np.float32)                                         # [B, O]

    # approx top-64 per row, then exact recompute of just those candidates
    # via the 32-entry COO rows (sum_p vals[o,p] * input[b, idx[o,p]])
    cand = np.argpartition(-S, NCAND - 1, axis=1)[:, :NCAND]   # [B, 64]
    rows = np.arange(B)[:, None]
    wi = weight_idx[cand]                                      # [B, 64, 32]
    wv = weight_vals[cand].astype(np.float64)                  # [B, 64, 32]
    xg = input[rows[:, :, None], wi]                           # [B, 64, 32]
    exact = (wv * xg).sum(axis=2)                              # [B, 64] f64

    # exact top-32 of the 64 (desc value, ties by lower column like top_k)
    order = np.lexsort((cand, -exact), axis=1)[:, :TOPK]
    g32 = np.take_along_axis(cand, order, axis=1)
    v32 = np.take_along_axis(exact, order, axis=1).astype(np.float32)

    out = np.zeros((B, O), dtype=np.float32)
    out[rows, g32] = v32
    return out


def kernel(input, weight_vals, weight_idx):
    in_maps = prepare_in_maps(input, weight_vals, weight_idx)
    res = run_bass_kernel_spmd(_get_program(), in_maps, list(range(NCORES)))
    return gather_output(res.results)
